# revision 17
# baseline (speedup 1.0000x reference)
"""2-layer GAT (heads=1) + linear classifier, distributed over 8 TRN2 NeuronCores.

v2 design (vs v1 baseline):
  - Per-(block, src-half) bulk row gather via gpsimd.dma_gather (one SWDGE
    instruction per ~1152 rows instead of one per 128) from an fp16 table of
    1152-elem rows [h(1024) | as | pad]; `as` rides the gathered row.
  - ad[dst] per edge via the staircase one-hot matmuls (edges sorted by dst);
    leaky-relu/exp batched per (block, half) on ACT.
  - Scatter-sum emitted transposed: numT[f, d] = sum_e m[e, f] * s[e, d] with
    the gathered rows as stationary operands -> edge-phase output lands
    directly in the lhsT layout the next matmul needs (no PE transposes).
  - AllGather split into src-half collectives (A: shard rows 0:640, B: rest)
    so each AG overlaps compute; layer-2 linear / classifier interleaved
    per-block with the edge phases.
"""
import sys

sys.path.insert(0, "/opt/trn_rl_repo")

import numpy as np

N_NODES = 10000
N_EDGES = 160000
F_IN, D, N_CLS = 300, 1024, 10
NEG_SLOPE = 0.2
NC_ = 8                      # cores
NPC = N_NODES // NC_         # real nodes per core (1250)
PADN = 1280                  # padded nodes per core (10 * 128)
NB = 10                      # dst blocks of 128 per core
HALF = PADN // 2             # 640 rows per src-half shard
TROW = 1152                  # table row: [h(1024) | as | pad(127)]
FKX = 3                      # 384 / 128 k-chunks for layer 1
DKX = D // 128               # 8 k-chunks for layer 2
F_PAD = FKX * 128            # 384

_CACHE = {}
USE_DMA_GATHER = True
SPLIT_AG = True


# ----------------------------------------------------------------- host prep
def _edge_metadata(src, dst):
    """Per-core, per-dst-block, per-src-half chunked edge schedule."""
    percore = []
    cnts = {"A": [], "B": []}
    for c in range(NC_):
        sel = (dst >= c * NPC) & (dst < (c + 1) * NPC)
        s_c = src[sel]
        d_c = dst[sel] - c * NPC
        blocks = []
        for b in range(NB):
            bsel = (d_c >= b * 128) & (d_c < (b + 1) * 128)
            sb, db = s_c[bsel], d_c[bsel]
            srow = sb % NPC           # local row within owner
            half = np.where(srow < HALF, 0, 1)
            hb = {}
            for h, nm in ((0, "A"), (1, "B")):
                hs = half == h
                s_h, d_h = sb[hs], db[hs]
                o = np.argsort(d_h, kind="stable")
                s_h, d_h = s_h[o], d_h[o]
                # gather row in the half-table: owner*640 + (row - h*640)
                grow = (s_h // NPC) * HALF + (s_h % NPC) - h * HALF
                hb[nm] = (grow.astype(np.int64), d_h.astype(np.int64))
                cnts[nm].append(len(s_h))
            blocks.append(hb)
        percore.append(blocks)
    KA = max(1, int(np.ceil(max(cnts["A"]) / 128)))
    KB = max(1, int(np.ceil(max(cnts["B"]) / 128)))
    assert KA * 128 <= 2048 and KB * 128 <= 2048
    metas = []
    for c in range(NC_):
        m = {}
        for nm, KH in (("A", KA), ("B", KB)):
            EBP = KH * 128
            idx16 = np.zeros((128, NB * KH * 8), np.int16)
            idx32 = np.zeros((128, NB * KH), np.int32)
            dstl = np.full((128, NB * KH), -1.0, np.float32)
            lo_a = np.zeros((128, NB), np.float32)
            hi_a = np.zeros((128, NB), np.float32)
            for b in range(NB):
                grow, d_h = percore[c][b][nm]
                cnt = len(grow)
                # slot j -> (p=j%128, k=j//128); idx16 uses the HW layout:
                # position i reads idx16[16 + i%16, 8*(i//128) + (i%128)//16]
                j = np.arange(cnt)
                col = b * KH * 8 + 8 * (j // 128) + (j % 128) // 16
                rowp = (j % 128) % 16
                idx16[16 + rowp, col] = grow               # HW layout
                idx16[j % 16, b * KH * 8 + j // 16] = grow  # sim layout
                idx32[j % 128, b * KH + j // 128] = grow
                dl = (d_h - b * 128).astype(np.float32)
                flat = np.full(EBP, -1.0, np.float32)
                flat[:cnt] = dl
                dstl[:, b * KH:(b + 1) * KH] = flat.reshape(KH, 128).T
                dloc = d_h - b * 128
                nreal = min(128, NPC - b * 128)
                for dd in range(nreal):
                    lo_a[dd, b] = np.searchsorted(dloc, dd, side="left")
                    hi_a[dd, b] = np.searchsorted(dloc, dd, side="right")
            m[nm] = dict(idx=idx16, idx32=idx32, dstl=dstl, lo=lo_a, hi=hi_a)
        metas.append(m)
    return KA, KB, metas


def _pack_inputs(x, edge_index, W1, att_src1, att_dst1, b1,
                 W2, att_src2, att_dst2, b2, fc_w, fc_b):
    f16, f32 = np.float16, np.float32
    src = np.concatenate([edge_index[0], np.arange(N_NODES)]).astype(np.int64)
    dst = np.concatenate([edge_index[1], np.arange(N_NODES)]).astype(np.int64)
    KA, KB, metas = _edge_metadata(src, dst)

    def fold_k(a, kx):  # [kx*128, m] -> [128, kx*m]
        kxp, m = a.shape
        return np.ascontiguousarray(
            a.reshape(kx, 128, m).transpose(1, 0, 2).reshape(128, kx * m))

    W1p = np.zeros((F_PAD, D), f32)
    W1p[:F_IN] = W1
    w1sd = np.stack([W1p @ att_src1, W1p @ att_dst1], axis=1)  # [384, 2]
    w2sd = np.stack([W2 @ att_src2, W2 @ att_dst2], axis=1)    # [1024, 2]
    W1f = fold_k(W1p.astype(f16), FKX)
    W2f = fold_k(W2.astype(f16), DKX)
    FCf = fold_k(fc_w.astype(f16), DKX)
    w1sdf = fold_k(w1sd.astype(f16), FKX)
    w2sdf = fold_k(w2sd.astype(f16), DKX)
    bT1 = np.ascontiguousarray(b1.astype(f32).reshape(DKX, 128).T)  # [128, DKX]
    bT2 = np.ascontiguousarray(b2.astype(f32).reshape(DKX, 128).T)
    fcb = np.broadcast_to(fc_b.astype(f32), (128, N_CLS)).copy()
    iotaF = np.broadcast_to(np.arange(128, dtype=f16), (128, 128)).copy()
    EBPM = max(KA, KB) * 128
    iotaE = np.broadcast_to(np.arange(EBPM, dtype=f16), (128, EBPM)).copy()

    in_maps = []
    for c in range(NC_):
        xs = np.zeros((PADN, F_PAD), f32)
        xs[:NPC, :F_IN] = x[c * NPC:(c + 1) * NPC]
        xTf = fold_k(np.ascontiguousarray(xs.T).astype(f16), FKX)
        mA, mB = metas[c]["A"], metas[c]["B"]
        in_maps.append({
            "xT": xTf, "W1": W1f, "W2": W2f, "FC": FCf,
            "w1sd": w1sdf, "w2sd": w2sdf,
            "bT1": bT1, "bT2": bT2, "fcb": fcb,
            "iotaF": iotaF, "iotaE": iotaE,
            "idxA": mA["idx"], "dstlA": mA["dstl"], "loA": mA["lo"], "hiA": mA["hi"],
            "idxB": mB["idx"], "dstlB": mB["dstl"], "loB": mB["lo"], "hiB": mB["hi"],
            "idx32A": mA["idx32"], "idx32B": mB["idx32"],
        })
    return (KA, KB), in_maps


# ------------------------------------------------------------- device program
def _build_program(key, reps=1, debug=False):
    KA, KB = key
    import concourse.bacc as bacc
    import concourse.bass as bass
    import concourse.mybir as mybir
    from concourse.tile import TileContext

    f32, f16, i16 = mybir.dt.float32, mybir.dt.float16, mybir.dt.int16
    AF = mybir.ActivationFunctionType
    ALU = mybir.AluOpType
    AX = mybir.AxisListType
    EBPM = max(KA, KB) * 128

    nc = bacc.Bacc("TRN2", target_bir_lowering=False, debug=False, num_devices=NC_)
    d_xT = nc.dram_tensor("xT", [128, FKX * PADN], f16, kind="ExternalInput")
    d_W1 = nc.dram_tensor("W1", [128, FKX * D], f16, kind="ExternalInput")
    d_W2 = nc.dram_tensor("W2", [128, DKX * D], f16, kind="ExternalInput")
    d_FC = nc.dram_tensor("FC", [128, DKX * N_CLS], f16, kind="ExternalInput")
    d_w1sd = nc.dram_tensor("w1sd", [128, FKX * 2], f16, kind="ExternalInput")
    d_w2sd = nc.dram_tensor("w2sd", [128, DKX * 2], f16, kind="ExternalInput")
    d_bT1 = nc.dram_tensor("bT1", [128, DKX], f32, kind="ExternalInput")
    d_bT2 = nc.dram_tensor("bT2", [128, DKX], f32, kind="ExternalInput")
    d_fcb = nc.dram_tensor("fcb", [128, N_CLS], f32, kind="ExternalInput")
    d_iotaF = nc.dram_tensor("iotaF", [128, 128], f16, kind="ExternalInput")
    d_iotaE = nc.dram_tensor("iotaE", [128, EBPM], f16, kind="ExternalInput")
    d_idx = {"A": nc.dram_tensor("idxA", [128, NB * KA * 8], i16, kind="ExternalInput"),
             "B": nc.dram_tensor("idxB", [128, NB * KB * 8], i16, kind="ExternalInput")}
    i32 = mybir.dt.int32
    d_idx32 = {"A": nc.dram_tensor("idx32A", [128, NB * KA], i32, kind="ExternalInput"),
               "B": nc.dram_tensor("idx32B", [128, NB * KB], i32, kind="ExternalInput")}
    d_dstl = {"A": nc.dram_tensor("dstlA", [128, NB * KA], f32, kind="ExternalInput"),
              "B": nc.dram_tensor("dstlB", [128, NB * KB], f32, kind="ExternalInput")}
    _ = None
    d_lo = {"A": nc.dram_tensor("loA", [128, NB], f32, kind="ExternalInput"),
            "B": nc.dram_tensor("loB", [128, NB], f32, kind="ExternalInput")}
    d_hi = {"A": nc.dram_tensor("hiA", [128, NB], f32, kind="ExternalInput"),
            "B": nc.dram_tensor("hiB", [128, NB], f32, kind="ExternalInput")}
    d_out = nc.dram_tensor("out", [PADN, N_CLS], f32, kind="ExternalOutput")

    # per-layer, per-half internal shards + shared gathered tables
    shard = {(l, h): nc.dram_tensor(f"shard{l}{h}", [HALF, TROW], f16)
             for l in (1, 2) for h in ("A", "B")}
    tab = {(l, h): nc.dram_tensor(f"tab{l}{h}", [NC_ * HALF, TROW], f16,
                                  addr_space="Shared")
           for l in (1, 2) for h in ("A", "B")}
    dbg = {}
    if debug:
        dbg["emb1"] = nc.dram_tensor("dbg_emb1", [128, PADN], f16,
                                     kind="ExternalOutput")
        dbg["sh1A"] = nc.dram_tensor("dbg_sh1A", [HALF, TROW], f16,
                                     kind="ExternalOutput")
        dbg["adc1"] = nc.dram_tensor("dbg_adc1", [128, NB], f32,
                                     kind="ExternalOutput")

    KH_ = {"A": KA, "B": KB}

    with TileContext(nc) as tc:
        with (
            tc.tile_pool(name="const", bufs=1) as cp,
            tc.tile_pool(name="embT", bufs=1) as ep,
            tc.tile_pool(name="work", bufs=2) as wp,
            tc.tile_pool(name="gath", bufs=2) as gp,
            tc.tile_pool(name="stair", bufs=2) as stp,
            tc.tile_pool(name="smat", bufs=12) as sp,
            tc.tile_pool(name="ps", bufs=1, space="PSUM") as ps,
        ):
            # ---- constants
            def cload(name, dd, dt):
                t = cp.tile([128, dd.shape[1]], dt, tag=name, name=name)
                nc.sync.dma_start(out=t[:], in_=dd[:])
                return t

            xT = cload("xT", d_xT, f16)
            W1 = cload("W1", d_W1, f16)
            W2 = cload("W2", d_W2, f16)
            FC = cload("FC", d_FC, f16)
            w1sd = cload("w1sd", d_w1sd, f16)
            w2sd = cload("w2sd", d_w2sd, f16)
            bT1 = cload("bT1", d_bT1, f32)
            bT2 = cload("bT2", d_bT2, f32)
            fcb = cload("fcb", d_fcb, f32)
            iotaF = cload("iotaF", d_iotaF, f16)
            iotaE = cload("iotaE", d_iotaE, f16)
            idx_t = {h: cload(f"idx{h}", d_idx[h], i16) for h in ("A", "B")}
            idx32_t = {h: cload(f"idx32{h}", d_idx32[h], i32) for h in ("A", "B")}
            dstl_t = {h: cload(f"dstl{h}", d_dstl[h], f32) for h in ("A", "B")}
            lo_t = {h: cload(f"lo{h}", d_lo[h], f32) for h in ("A", "B")}
            hi_t = {h: cload(f"hi{h}", d_hi[h], f32) for h in ("A", "B")}
            ones_c = cp.tile([128, 1], f16)
            nc.gpsimd.memset(ones_c[:], 1.0)
            ones_r = cp.tile([128, 128], f32)
            nc.gpsimd.memset(ones_r[:], 1.0)

            # embT tiles: [128 f, PADN nodes] per k-chunk; mid = layer1 out,
            # fin = layer2 out
            embM = [ep.tile([128, PADN], f16, tag=f"embM{kc}", name=f"embM{kc}")
                    for kc in range(DKX)]
            embF = embM  # layer-1 copies are dead once linear-2 consumed them
            adcol = {1: cp.tile([128, NB], f16, tag="adcol1", name="adcol1"),
                     2: cp.tile([128, NB], f16, tag="adcol2", name="adcol2")}

            # zero the gather tiles once (stale bytes in skipped slots must be
            # finite; thereafter they only ever hold previously gathered rows)
            for h in ("A", "B"):
                for _ in range(2):
                    t = gp.tile([128, KH_[h], TROW], f16, tag=f"m{h}",
                                name=f"minit{h}")
                    nc.vector.memset(t[:], 0.0)

            def linear(layer, mt, lhs):
                """One 128-row block of h = in @ W (+as/ad); write shard rows."""
                kx = FKX if layer == 1 else DKX
                Wt = W1 if layer == 1 else W2
                wsd = w1sd if layer == 1 else w2sd
                stage = wp.tile([128, TROW], f16, tag="stage")
                for half in range(2):
                    hh = ps.tile([128, 512], f32, tag="bank5", name="hh", bufs=2)
                    for kc in range(kx):
                        nc.tensor.matmul(
                            hh[:], lhsT=lhs(kc),
                            rhs=Wt[:, kc * D + half * 512: kc * D + half * 512 + 512],
                            start=(kc == 0), stop=(kc == kx - 1))
                    nc.scalar.activation(
                        stage[:, half * 512:half * 512 + 512], hh[:], AF.Copy)
                pasd = ps.tile([128, 128], f32, tag="denls", name="pasd", bufs=2)
                pasd = pasd[:, 0:2]
                for kc in range(kx):
                    nc.tensor.matmul(pasd[:], lhsT=lhs(kc),
                                     rhs=wsd[:, 2 * kc:2 * kc + 2],
                                     start=(kc == 0), stop=(kc == kx - 1))
                nc.vector.tensor_copy(out=stage[:, 1024:1025], in_=pasd[:, 0:1])
                nc.vector.memset(stage[:, 1025:TROW], 0.0)
                nc.vector.tensor_copy(out=adcol[layer][:, mt:mt + 1],
                                      in_=pasd[:, 1:2])
                h = "A" if mt < 5 else "B"
                r0 = (mt - (0 if mt < 5 else 5)) * 128
                nc.sync.dma_start(out=shard[(layer, h)][r0:r0 + 128, :],
                                  in_=stage[:])

            def allgather(layer, h):
                nc.gpsimd.collective_compute(
                    "AllGather", mybir.AluOpType.bypass,
                    replica_groups=[list(range(NC_))],
                    ins=[shard[(layer, h)][:]], outs=[tab[(layer, h)][:]])

            def edge_block(layer, b, embT_out, bT):
                """Softmax-weighted aggregation for dst block b -> embT_out."""
                numT = ps.tile([128, DKX, 128], f32, tag="numT", name="numT",
                               bufs=2)
                psmall = ps.tile([128, 512], f32, tag="bank5", name="psmall",
                                 bufs=2)
                pden = ps.tile([128, 128], f32, tag="denls", name="pden", bufs=2)
                den = pden[:, 0:128]
                rbc_ps = psmall[:, 128:256]
                for h in ("A", "B"):
                    KH = KH_[h]
                    EBP = KH * 128
                    m = gp.tile([128, KH, TROW], f16, tag=f"m{h}", name=f"m{h}")
                    if USE_DMA_GATHER:
                        # ring is 128 descs/engine; half-KH pieces (<=40
                        # descs) keep ~3 pieces in flight on the queue
                        pc = 3
                        for k0 in range(0, KH, pc):
                            kn = min(pc, KH - k0)
                            nc.gpsimd.dma_gather(
                                out_ap=m[:, k0:k0 + kn, :],
                                in_ap=tab[(layer, h)][:],
                                idxs_ap=idx_t[h][:, (b * KH + k0) * 8:
                                                 (b * KH + k0 + kn) * 8],
                                num_idxs=kn * 128, num_idxs_reg=kn * 128,
                                elem_size=TROW)
                    else:
                        for k in range(KH):
                            nc.gpsimd.indirect_dma_start(
                                out=m[:, k, :], out_offset=None,
                                in_=tab[(layer, h)][:],
                                in_offset=bass.IndirectOffsetOnAxis(
                                    ap=idx32_t[h][:, b * KH + k:b * KH + k + 1],
                                    axis=0))
                    aA = stp.tile([128, EBP], f16, tag=f"aA{h}")
                    nc.vector.tensor_scalar(out=aA[:], in0=iotaE[:, 0:EBP],
                                            scalar1=lo_t[h][:, b:b + 1],
                                            scalar2=None, op0=ALU.is_ge)
                    aB = stp.tile([128, EBP], f16, tag=f"aB{h}")
                    nc.vector.tensor_scalar(out=aB[:], in0=iotaE[:, 0:EBP],
                                            scalar1=hi_t[h][:, b:b + 1],
                                            scalar2=None, op0=ALU.is_ge)
                    s0t = stp.tile([128, EBP], f16, tag=f"s0t{h}")
                    nc.vector.tensor_tensor(out=s0t[:], in0=aA[:], in1=aB[:],
                                            op=ALU.subtract)
                    padg = psmall[:, 256 + (0 if h == "A" else 128):
                                  256 + (0 if h == "A" else 128) + KH]

                    for k in range(KH):
                        nc.tensor.matmul(padg[:, k:k + 1],
                                         lhsT=s0t[:, k * 128:(k + 1) * 128],
                                         rhs=adcol[layer][:, b:b + 1],
                                         start=True, stop=True)
                    adg = wp.tile([128, KH], f32, tag=f"adg{h}")
                    nc.vector.tensor_copy(out=adg[:], in_=padg[:])
                    z = wp.tile([128, KH], f32, tag=f"z{h}")
                    nc.vector.tensor_tensor(out=z[:], in0=adg[:],
                                            in1=m[:, :, 1024], op=ALU.add)
                    zs = wp.tile([128, KH], f32, tag=f"zs{h}")
                    nc.vector.tensor_scalar(out=zs[:], in0=z[:],
                                            scalar1=NEG_SLOPE, scalar2=None,
                                            op0=ALU.mult)
                    ee = wp.tile([128, KH], f32, tag=f"ee{h}")
                    nc.vector.tensor_tensor(out=ee[:], in0=z[:], in1=zs[:],
                                            op=ALU.max)
                    ex = wp.tile([128, KH], f32, tag=f"ex{h}")
                    nc.scalar.activation(ex[:], ee[:], AF.Exp)
                    for k in range(KH):
                        c = b * KH + k
                        s0 = sp.tile([128, 128], f16, tag="s0")
                        nc.vector.tensor_scalar(out=s0[:], in0=iotaF[:],
                                                scalar1=dstl_t[h][:, c:c + 1],
                                                scalar2=None, op0=ALU.is_equal)
                        s = sp.tile([128, 128], f16, tag="s")
                        nc.vector.tensor_scalar(out=s[:], in0=s0[:],
                                                scalar1=ex[:, k:k + 1],
                                                scalar2=None, op0=ALU.mult)
                        st = (h == "A" and k == 0)
                        sp_ = (h == "B" and k == KB - 1)
                        for kc in range(DKX):
                            nc.tensor.matmul(
                                numT[:, kc, :],
                                lhsT=m[:, k, kc * 128:(kc + 1) * 128],
                                rhs=s[:], start=(st and kc % 4 == 0),
                                stop=(sp_ and kc % 4 == 3))
                        nc.tensor.matmul(den[0:1, :], lhsT=ones_c[:], rhs=s[:],
                                         start=st, stop=sp_)
                # normalize + bias + relu into embT_out (features on partitions)
                dn = wp.tile([128, 128], f32, tag="dn")
                nc.vector.tensor_scalar(out=dn[0:1, :], in0=den[0:1, :],
                                        scalar1=1e-30, scalar2=None, op0=ALU.max)
                rc = wp.tile([128, 128], f32, tag="rc")
                nc.vector.reciprocal(out=rc[0:1, :], in_=dn[0:1, :])
                nc.tensor.matmul(rbc_ps[:, :], lhsT=ones_r[0:1, :],
                                 rhs=rc[0:1, :], start=True, stop=True)
                rbc = wp.tile([128, 128], f32, tag="rbc_sb")
                nc.vector.tensor_copy(out=rbc[:], in_=rbc_ps[:, :])
                tsc = wp.tile([128, DKX, 128], f32, tag="tsc")
                nc.vector.tensor_tensor(
                    out=tsc[:], in0=numT[:],
                    in1=rbc[:, None, :].broadcast_to([128, DKX, 128]),
                    op=ALU.mult)
                for kc in range(DKX):
                    nc.scalar.activation(
                        embT_out[kc][:, b * 128:(b + 1) * 128], tsc[:, kc, :],
                        AF.Relu, bias=bT[:, kc:kc + 1], scale=1.0)

            def classifier(b):
                pl = ps.tile([128, 128], f32, tag="denls", name="plog", bufs=2)
                pl = pl[:, 0:N_CLS]
                for kc in range(DKX):
                    nc.tensor.matmul(
                        pl[:], lhsT=embF[kc][:, b * 128:(b + 1) * 128],
                        rhs=FC[:, kc * N_CLS:(kc + 1) * N_CLS],
                        start=(kc == 0), stop=(kc == DKX - 1))
                lg = wp.tile([128, N_CLS], f32, tag="lg")
                nc.vector.tensor_tensor(out=lg[:], in0=pl[:], in1=fcb[:],
                                        op=ALU.add)
                mx = wp.tile([128, 1], f32, tag="mx")
                nc.vector.tensor_reduce(out=mx[:], in_=lg[:], axis=AX.X,
                                        op=ALU.max)
                lgs = wp.tile([128, N_CLS], f32, tag="lgs")
                nc.vector.tensor_scalar(out=lgs[:], in0=lg[:],
                                        scalar1=mx[:, 0:1], scalar2=None,
                                        op0=ALU.subtract)
                eg = wp.tile([128, N_CLS], f32, tag="eg")
                nc.scalar.activation(eg[:], lgs[:], AF.Exp)
                sm = wp.tile([128, 1], f32, tag="sm")
                nc.vector.tensor_reduce(out=sm[:], in_=eg[:], axis=AX.X,
                                        op=ALU.add)
                rcs = wp.tile([128, 1], f32, tag="rcs")
                nc.vector.reciprocal(out=rcs[:], in_=sm[:])
                ot = wp.tile([128, N_CLS], f32, tag="ot")
                nc.vector.tensor_scalar(out=ot[:], in0=eg[:],
                                        scalar1=rcs[:, 0:1], scalar2=None,
                                        op0=ALU.mult)
                nc.sync.dma_start(out=d_out[b * 128:(b + 1) * 128, :], in_=ot[:])

            for _ in range(reps):
                # ---- layer 1 linear + split AllGather
                for mt in range(NB):
                    def lhs1(kc, mt=mt):
                        return xT[:, kc * PADN + mt * 128: kc * PADN + (mt + 1) * 128]
                    linear(1, mt, lhs1)
                    if mt == 4 and SPLIT_AG:
                        allgather(1, "A")
                if not SPLIT_AG:
                    allgather(1, "A")
                allgather(1, "B")
                # ---- edge layer 1 interleaved with layer 2 linear
                for b in range(NB):
                    edge_block(1, b, embM, bT1)

                    def lhs2(kc, b=b):
                        return embM[kc][:, b * 128:(b + 1) * 128]
                    linear(2, b, lhs2)
                    if b == 4 and SPLIT_AG:
                        allgather(2, "A")
                if not SPLIT_AG:
                    allgather(2, "A")
                allgather(2, "B")
                # ---- edge layer 2 interleaved with classifier
                for b in range(NB):
                    edge_block(2, b, embF, bT2)
                    classifier(b)
                if debug:
                    nc.sync.dma_start(out=dbg["emb1"][:], in_=embM[0][:])
                    nc.gpsimd.dma_start(out=dbg["sh1A"][:], in_=shard[(1, "A")][:])
                    ac = wp.tile([128, NB], f32, tag="dbgac")
                    nc.vector.tensor_copy(out=ac[:], in_=adcol[1][:])
                    nc.sync.dma_start(out=dbg["adc1"][:], in_=ac[:])
    nc.compile()
    return nc


# ------------------------------------------------------------------ execution
class _Runner:
    """Cached-jit SPMD executor (axon/PJRT path of run_bass_kernel_spmd)."""

    def __init__(self, nc, n_cores=NC_):
        import jax
        from jax.sharding import Mesh, PartitionSpec
        from jax.experimental.shard_map import shard_map
        import concourse.mybir as mybir
        from concourse.bass2jax import (_bass_exec_p, install_neuronx_cc_hook,
                                        partition_id_tensor)

        install_neuronx_cc_hook()
        self.jax = jax
        self.n_cores = n_cores
        pname = nc.partition_id_tensor.name if nc.partition_id_tensor else None
        in_names, out_names, out_avals, zero_outs = [], [], [], []
        for alloc in nc.m.functions[0].allocations:
            if not isinstance(alloc, mybir.MemoryLocationSet):
                continue
            name = alloc.memorylocations[0].name
            if alloc.kind == "ExternalInput":
                if name != pname:
                    in_names.append(name)
            elif alloc.kind == "ExternalOutput":
                shape = tuple(alloc.tensor_shape)
                dtype = mybir.dt.np(alloc.dtype)
                out_names.append(name)
                out_avals.append(jax.core.ShapedArray(shape, dtype))
                zero_outs.append(np.zeros(shape, dtype))
        self.in_names, self.out_names = in_names, out_names
        self.out_avals, self.zero_outs = out_avals, zero_outs
        n_params = len(in_names)
        all_in = list(in_names) + list(out_names)
        if pname is not None:
            all_in.append(pname)

        def _body(*args):
            operands = list(args)
            if pname is not None:
                operands.append(partition_id_tensor())
            return tuple(_bass_exec_p.bind(
                *operands, out_avals=tuple(out_avals), in_names=tuple(all_in),
                out_names=tuple(out_names), lowering_input_output_aliases=(),
                sim_require_finite=True, sim_require_nnan=True, nc=nc))

        devices = jax.devices()[:n_cores]
        self.mesh = Mesh(np.asarray(devices), ("core",))
        n_outs = len(out_names)
        in_specs = (PartitionSpec("core"),) * (n_params + n_outs)
        out_specs = (PartitionSpec("core"),) * n_outs
        self.sharded = jax.jit(
            shard_map(_body, mesh=self.mesh, in_specs=in_specs,
                      out_specs=out_specs, check_rep=False),
            keep_unused=True)
        self.PartitionSpec = PartitionSpec

    def set_inputs(self, in_maps):
        jax = self.jax
        n = self.n_cores
        concat_in = [
            np.concatenate([np.asarray(in_maps[c][k]) for c in range(n)], axis=0)
            for k in self.in_names]
        concat_zero = [np.zeros((n * z.shape[0], *z.shape[1:]), z.dtype)
                       for z in self.zero_outs]
        sh = jax.sharding.NamedSharding(self.mesh, self.PartitionSpec("core"))
        self._args = [jax.device_put(a, sh) for a in concat_in + concat_zero]
        jax.block_until_ready(self._args)

    def run(self):
        return self.sharded(*self._args)

    def run_np(self):
        outs = self.jax.block_until_ready(self.run())
        n = self.n_cores
        return [
            {k: np.asarray(outs[i]).reshape(n, *self.out_avals[i].shape)[c]
             for i, k in enumerate(self.out_names)}
            for c in range(n)]


def _get_runner(key, reps=1, debug=False):
    ck = (key, reps, debug, USE_DMA_GATHER, SPLIT_AG)
    if ck not in _CACHE:
        nc = _build_program(key, reps, debug)
        _CACHE[ck] = _Runner(nc)
    return _CACHE[ck]


def kernel(**inputs):
    inputs = {k: np.asarray(v) for k, v in inputs.items()}
    assert inputs["x"].shape == (N_NODES, F_IN)
    assert inputs["edge_index"].shape == (2, N_EDGES)
    key, in_maps = _pack_inputs(**inputs)
    runner = _get_runner(key)
    runner.set_inputs(in_maps)
    res = runner.run_np()
    out = np.concatenate([res[c]["out"][:NPC] for c in range(NC_)], axis=0)
    return out.astype(np.float32)


# revision 18
# speedup vs baseline: 86.0377x; 86.0377x over previous
"""2-layer GAT (heads=1) + linear classifier, distributed over 8 TRN2 NeuronCores.

v2 design (vs v1 baseline):
  - Per-(block, src-half) bulk row gather via gpsimd.dma_gather (one SWDGE
    instruction per ~1152 rows instead of one per 128) from an fp16 table of
    1152-elem rows [h(1024) | as | pad]; `as` rides the gathered row.
  - ad[dst] per edge via the staircase one-hot matmuls (edges sorted by dst);
    leaky-relu/exp batched per (block, half) on ACT.
  - Scatter-sum emitted transposed: numT[f, d] = sum_e m[e, f] * s[e, d] with
    the gathered rows as stationary operands -> edge-phase output lands
    directly in the lhsT layout the next matmul needs (no PE transposes).
  - AllGather split into src-half collectives (A: shard rows 0:640, B: rest)
    so each AG overlaps compute; layer-2 linear / classifier interleaved
    per-block with the edge phases.
"""
import sys

sys.path.insert(0, "/opt/trn_rl_repo")

import numpy as np

N_NODES = 10000
N_EDGES = 160000
F_IN, D, N_CLS = 300, 1024, 10
NEG_SLOPE = 0.2
NC_ = 8                      # cores
NPC = N_NODES // NC_         # real nodes per core (1250)
PADN = 1280                  # padded nodes per core (10 * 128)
NB = 10                      # dst blocks of 128 per core
HALF = PADN // 2             # 640 rows per src-half shard
TROW = 1152                  # table row: [h(1024) | as | pad(127)]
FKX = 3                      # 384 / 128 k-chunks for layer 1
DKX = D // 128               # 8 k-chunks for layer 2
F_PAD = FKX * 128            # 384

_CACHE = {}
USE_DMA_GATHER = True
SPLIT_AG = True


# ----------------------------------------------------------------- host prep
def _edge_metadata(src, dst):
    """Per-core, per-dst-block, per-src-half chunked edge schedule."""
    percore = []
    cnts = {"A": [], "B": []}
    for c in range(NC_):
        sel = (dst >= c * NPC) & (dst < (c + 1) * NPC)
        s_c = src[sel]
        d_c = dst[sel] - c * NPC
        blocks = []
        for b in range(NB):
            bsel = (d_c >= b * 128) & (d_c < (b + 1) * 128)
            sb, db = s_c[bsel], d_c[bsel]
            srow = sb % NPC           # local row within owner
            half = np.where(srow < HALF, 0, 1)
            hb = {}
            for h, nm in ((0, "A"), (1, "B")):
                hs = half == h
                s_h, d_h = sb[hs], db[hs]
                o = np.argsort(d_h, kind="stable")
                s_h, d_h = s_h[o], d_h[o]
                # gather row in the half-table: owner*640 + (row - h*640)
                grow = (s_h // NPC) * HALF + (s_h % NPC) - h * HALF
                hb[nm] = (grow.astype(np.int64), d_h.astype(np.int64))
                cnts[nm].append(len(s_h))
            blocks.append(hb)
        percore.append(blocks)
    KA = max(1, int(np.ceil(max(cnts["A"]) / 128)))
    KB = max(1, int(np.ceil(max(cnts["B"]) / 128)))
    assert KA * 128 <= 2048 and KB * 128 <= 2048
    metas = []
    for c in range(NC_):
        m = {}
        for nm, KH in (("A", KA), ("B", KB)):
            EBP = KH * 128
            idx16 = np.zeros((128, NB * KH * 8), np.int16)
            idx32 = np.zeros((128, NB * KH), np.int32)
            dstl = np.full((128, NB * KH), -1.0, np.float32)
            lo_a = np.zeros((128, NB), np.float32)
            hi_a = np.zeros((128, NB), np.float32)
            for b in range(NB):
                grow, d_h = percore[c][b][nm]
                cnt = len(grow)
                # slot j -> (p=j%128, k=j//128); idx16 uses the HW layout:
                # position i reads idx16[16 + i%16, 8*(i//128) + (i%128)//16]
                j = np.arange(cnt)
                col = b * KH * 8 + 8 * (j // 128) + (j % 128) // 16
                rowp = (j % 128) % 16
                idx16[16 + rowp, col] = grow               # HW layout
                idx16[j % 16, b * KH * 8 + j // 16] = grow  # sim layout
                idx32[j % 128, b * KH + j // 128] = grow
                dl = (d_h - b * 128).astype(np.float32)
                flat = np.full(EBP, -1.0, np.float32)
                flat[:cnt] = dl
                dstl[:, b * KH:(b + 1) * KH] = flat.reshape(KH, 128).T
                dloc = d_h - b * 128
                nreal = min(128, NPC - b * 128)
                for dd in range(nreal):
                    lo_a[dd, b] = np.searchsorted(dloc, dd, side="left")
                    hi_a[dd, b] = np.searchsorted(dloc, dd, side="right")
            m[nm] = dict(idx=idx16, idx32=idx32, dstl=dstl, lo=lo_a, hi=hi_a)
        metas.append(m)
    return KA, KB, metas


def _pack_inputs(x, edge_index, W1, att_src1, att_dst1, b1,
                 W2, att_src2, att_dst2, b2, fc_w, fc_b):
    f16, f32 = np.float16, np.float32
    src = np.concatenate([edge_index[0], np.arange(N_NODES)]).astype(np.int64)
    dst = np.concatenate([edge_index[1], np.arange(N_NODES)]).astype(np.int64)
    KA, KB, metas = _edge_metadata(src, dst)

    def fold_k(a, kx):  # [kx*128, m] -> [128, kx*m]
        kxp, m = a.shape
        return np.ascontiguousarray(
            a.reshape(kx, 128, m).transpose(1, 0, 2).reshape(128, kx * m))

    W1p = np.zeros((F_PAD, D), f32)
    W1p[:F_IN] = W1
    w1sd = np.stack([W1p @ att_src1, W1p @ att_dst1], axis=1)  # [384, 2]
    w2sd = np.stack([W2 @ att_src2, W2 @ att_dst2], axis=1)    # [1024, 2]
    W1f = fold_k(W1p.astype(f16), FKX)
    W2f = fold_k(W2.astype(f16), DKX)
    FCf = fold_k(fc_w.astype(f16), DKX)
    w1sdf = fold_k(w1sd.astype(f16), FKX)
    w2sdf = fold_k(w2sd.astype(f16), DKX)
    bT1 = np.ascontiguousarray(b1.astype(f32).reshape(DKX, 128).T)  # [128, DKX]
    bT2 = np.ascontiguousarray(b2.astype(f32).reshape(DKX, 128).T)
    fcb = np.broadcast_to(fc_b.astype(f32), (128, N_CLS)).copy()
    iotaF = np.broadcast_to(np.arange(128, dtype=f16), (128, 128)).copy()
    EBPM = max(KA, KB) * 128
    iotaE = np.broadcast_to(np.arange(EBPM, dtype=f16), (128, EBPM)).copy()

    in_maps = []
    for c in range(NC_):
        xs = np.zeros((PADN, F_PAD), f32)
        xs[:NPC, :F_IN] = x[c * NPC:(c + 1) * NPC]
        xTf = fold_k(np.ascontiguousarray(xs.T).astype(f16), FKX)
        mA, mB = metas[c]["A"], metas[c]["B"]
        in_maps.append({
            "xT": xTf, "W1": W1f, "W2": W2f, "FC": FCf,
            "w1sd": w1sdf, "w2sd": w2sdf,
            "bT1": bT1, "bT2": bT2, "fcb": fcb,
            "iotaF": iotaF, "iotaE": iotaE,
            "idxA": mA["idx"], "dstlA": mA["dstl"], "loA": mA["lo"], "hiA": mA["hi"],
            "idxB": mB["idx"], "dstlB": mB["dstl"], "loB": mB["lo"], "hiB": mB["hi"],
            "idx32A": mA["idx32"], "idx32B": mB["idx32"],
        })
    return (KA, KB), in_maps


# ------------------------------------------------------------- device program
def _build_program(key, reps=1, debug=False):
    KA, KB = key
    import concourse.bacc as bacc
    import concourse.bass as bass
    import concourse.mybir as mybir
    from concourse.tile import TileContext

    f32, f16, i16 = mybir.dt.float32, mybir.dt.float16, mybir.dt.int16
    AF = mybir.ActivationFunctionType
    ALU = mybir.AluOpType
    AX = mybir.AxisListType
    EBPM = max(KA, KB) * 128

    nc = bacc.Bacc("TRN2", target_bir_lowering=False, debug=False, num_devices=NC_)
    d_xT = nc.dram_tensor("xT", [128, FKX * PADN], f16, kind="ExternalInput")
    d_W1 = nc.dram_tensor("W1", [128, FKX * D], f16, kind="ExternalInput")
    d_W2 = nc.dram_tensor("W2", [128, DKX * D], f16, kind="ExternalInput")
    d_FC = nc.dram_tensor("FC", [128, DKX * N_CLS], f16, kind="ExternalInput")
    d_w1sd = nc.dram_tensor("w1sd", [128, FKX * 2], f16, kind="ExternalInput")
    d_w2sd = nc.dram_tensor("w2sd", [128, DKX * 2], f16, kind="ExternalInput")
    d_bT1 = nc.dram_tensor("bT1", [128, DKX], f32, kind="ExternalInput")
    d_bT2 = nc.dram_tensor("bT2", [128, DKX], f32, kind="ExternalInput")
    d_fcb = nc.dram_tensor("fcb", [128, N_CLS], f32, kind="ExternalInput")
    d_iotaF = nc.dram_tensor("iotaF", [128, 128], f16, kind="ExternalInput")
    d_iotaE = nc.dram_tensor("iotaE", [128, EBPM], f16, kind="ExternalInput")
    d_idx = {"A": nc.dram_tensor("idxA", [128, NB * KA * 8], i16, kind="ExternalInput"),
             "B": nc.dram_tensor("idxB", [128, NB * KB * 8], i16, kind="ExternalInput")}
    i32 = mybir.dt.int32
    d_idx32 = {"A": nc.dram_tensor("idx32A", [128, NB * KA], i32, kind="ExternalInput"),
               "B": nc.dram_tensor("idx32B", [128, NB * KB], i32, kind="ExternalInput")}
    d_dstl = {"A": nc.dram_tensor("dstlA", [128, NB * KA], f32, kind="ExternalInput"),
              "B": nc.dram_tensor("dstlB", [128, NB * KB], f32, kind="ExternalInput")}
    _ = None
    d_lo = {"A": nc.dram_tensor("loA", [128, NB], f32, kind="ExternalInput"),
            "B": nc.dram_tensor("loB", [128, NB], f32, kind="ExternalInput")}
    d_hi = {"A": nc.dram_tensor("hiA", [128, NB], f32, kind="ExternalInput"),
            "B": nc.dram_tensor("hiB", [128, NB], f32, kind="ExternalInput")}
    d_out = nc.dram_tensor("out", [PADN, N_CLS], f32, kind="ExternalOutput")

    # per-layer, per-half internal shards + shared gathered tables
    shard = {(l, h): nc.dram_tensor(f"shard{l}{h}", [HALF, TROW], f16)
             for l in (1, 2) for h in ("A", "B")}
    tab = {(l, h): nc.dram_tensor(f"tab{l}{h}", [NC_ * HALF, TROW], f16,
                                  addr_space="Shared")
           for l in (1, 2) for h in ("A", "B")}
    dbg = {}
    if debug:
        dbg["emb1"] = nc.dram_tensor("dbg_emb1", [128, PADN], f16,
                                     kind="ExternalOutput")
        dbg["sh1A"] = nc.dram_tensor("dbg_sh1A", [HALF, TROW], f16,
                                     kind="ExternalOutput")
        dbg["adc1"] = nc.dram_tensor("dbg_adc1", [128, NB], f32,
                                     kind="ExternalOutput")

    KH_ = {"A": KA, "B": KB}

    with TileContext(nc) as tc:
        with (
            tc.tile_pool(name="const", bufs=1) as cp,
            tc.tile_pool(name="embT", bufs=1) as ep,
            tc.tile_pool(name="work", bufs=2) as wp,
            tc.tile_pool(name="gath", bufs=2) as gp,
            tc.tile_pool(name="stair", bufs=1) as stp,
            tc.tile_pool(name="smat", bufs=8) as sp,
            tc.tile_pool(name="ps", bufs=1, space="PSUM") as ps,
        ):
            # ---- constants
            def cload(name, dd, dt):
                t = cp.tile([128, dd.shape[1]], dt, tag=name, name=name)
                nc.sync.dma_start(out=t[:], in_=dd[:])
                return t

            xT = cload("xT", d_xT, f16)
            W1 = cload("W1", d_W1, f16)
            W2 = cload("W2", d_W2, f16)
            FC = cload("FC", d_FC, f16)
            w1sd = cload("w1sd", d_w1sd, f16)
            w2sd = cload("w2sd", d_w2sd, f16)
            bT1 = cload("bT1", d_bT1, f32)
            bT2 = cload("bT2", d_bT2, f32)
            fcb = cload("fcb", d_fcb, f32)
            iotaF = cload("iotaF", d_iotaF, f16)
            iotaE = cload("iotaE", d_iotaE, f16)
            idx_t = {h: cload(f"idx{h}", d_idx[h], i16) for h in ("A", "B")}
            idx32_t = {h: cload(f"idx32{h}", d_idx32[h], i32) for h in ("A", "B")}
            dstl_t = {h: cload(f"dstl{h}", d_dstl[h], f32) for h in ("A", "B")}
            lo_t = {h: cload(f"lo{h}", d_lo[h], f32) for h in ("A", "B")}
            hi_t = {h: cload(f"hi{h}", d_hi[h], f32) for h in ("A", "B")}
            ones_c = cp.tile([128, 1], f16)
            nc.gpsimd.memset(ones_c[:], 1.0)
            ones_r = cp.tile([128, 128], f32)
            nc.gpsimd.memset(ones_r[:], 1.0)

            # embT tiles: [128 f, PADN nodes] per k-chunk; mid = layer1 out,
            # fin = layer2 out
            embM = [ep.tile([128, PADN], f16, tag=f"embM{kc}", name=f"embM{kc}")
                    for kc in range(DKX)]
            embF = embM  # layer-1 copies are dead once linear-2 consumed them
            adcol = {1: cp.tile([128, NB], f16, tag="adcol1", name="adcol1"),
                     2: cp.tile([128, NB], f16, tag="adcol2", name="adcol2")}

            # zero the gather tiles once (stale bytes in skipped slots must be
            # finite; thereafter they only ever hold previously gathered rows)
            for h in ("A", "B"):
                for _ in range(2):
                    t = gp.tile([128, KH_[h], TROW], f16, tag=f"m{h}",
                                name=f"minit{h}")
                    nc.vector.memset(t[:], 0.0)

            def linear(layer, mt, lhs):
                """One 128-row block of h = in @ W (+as/ad); write shard rows."""
                kx = FKX if layer == 1 else DKX
                Wt = W1 if layer == 1 else W2
                wsd = w1sd if layer == 1 else w2sd
                stage = wp.tile([128, TROW], f16, tag="stage")
                for half in range(2):
                    hh = ps.tile([128, 512], f32, tag="bank5", name="hh", bufs=2)
                    for kc in range(kx):
                        nc.tensor.matmul(
                            hh[:], lhsT=lhs(kc),
                            rhs=Wt[:, kc * D + half * 512: kc * D + half * 512 + 512],
                            start=(kc == 0), stop=(kc == kx - 1))
                    nc.scalar.activation(
                        stage[:, half * 512:half * 512 + 512], hh[:], AF.Copy)
                pasd = ps.tile([128, 128], f32, tag="denls", name="pasd", bufs=2)
                pasd = pasd[:, 0:2]
                for kc in range(kx):
                    nc.tensor.matmul(pasd[:], lhsT=lhs(kc),
                                     rhs=wsd[:, 2 * kc:2 * kc + 2],
                                     start=(kc == 0), stop=(kc == kx - 1))
                nc.vector.tensor_copy(out=stage[:, 1024:1025], in_=pasd[:, 0:1])
                nc.vector.memset(stage[:, 1025:TROW], 0.0)
                nc.vector.tensor_copy(out=adcol[layer][:, mt:mt + 1],
                                      in_=pasd[:, 1:2])
                h = "A" if mt < 5 else "B"
                r0 = (mt - (0 if mt < 5 else 5)) * 128
                nc.sync.dma_start(out=shard[(layer, h)][r0:r0 + 128, :],
                                  in_=stage[:])

            def allgather(layer, h):
                nc.gpsimd.collective_compute(
                    "AllGather", mybir.AluOpType.bypass,
                    replica_groups=[list(range(NC_))],
                    ins=[shard[(layer, h)][:]], outs=[tab[(layer, h)][:]])

            def edge_block(layer, b, embT_out, bT):
                """Softmax-weighted aggregation for dst block b -> embT_out."""
                numT = ps.tile([128, DKX, 128], f32, tag="numT", name="numT",
                               bufs=2)
                psmall = ps.tile([128, 512], f32, tag="bank5", name="psmall",
                                 bufs=2)
                pden = ps.tile([128, 128], f32, tag="denls", name="pden", bufs=2)
                den = pden[:, 0:128]
                rbc_ps = psmall[:, 128:256]
                for h in ("A", "B"):
                    KH = KH_[h]
                    EBP = KH * 128
                    m = gp.tile([128, KH, TROW], f16, tag=f"m{h}", name=f"m{h}")
                    if USE_DMA_GATHER:
                        # ring is 128 descs/engine; half-KH pieces (<=40
                        # descs) keep ~3 pieces in flight on the queue
                        pc = (KH + 1) // 2
                        for k0 in range(0, KH, pc):
                            kn = min(pc, KH - k0)
                            nc.gpsimd.dma_gather(
                                out_ap=m[:, k0:k0 + kn, :],
                                in_ap=tab[(layer, h)][:],
                                idxs_ap=idx_t[h][:, (b * KH + k0) * 8:
                                                 (b * KH + k0 + kn) * 8],
                                num_idxs=kn * 128, num_idxs_reg=kn * 128,
                                elem_size=TROW)
                    else:
                        for k in range(KH):
                            nc.gpsimd.indirect_dma_start(
                                out=m[:, k, :], out_offset=None,
                                in_=tab[(layer, h)][:],
                                in_offset=bass.IndirectOffsetOnAxis(
                                    ap=idx32_t[h][:, b * KH + k:b * KH + k + 1],
                                    axis=0))
                    aA = stp.tile([128, EBP], f16, tag=f"aA{h}")
                    nc.vector.tensor_scalar(out=aA[:], in0=iotaE[:, 0:EBP],
                                            scalar1=lo_t[h][:, b:b + 1],
                                            scalar2=None, op0=ALU.is_ge)
                    aB = stp.tile([128, EBP], f16, tag=f"aB{h}")
                    nc.vector.tensor_scalar(out=aB[:], in0=iotaE[:, 0:EBP],
                                            scalar1=hi_t[h][:, b:b + 1],
                                            scalar2=None, op0=ALU.is_ge)
                    s0t = stp.tile([128, EBP], f16, tag=f"s0t{h}")
                    nc.vector.tensor_tensor(out=s0t[:], in0=aA[:], in1=aB[:],
                                            op=ALU.subtract)
                    padg = psmall[:, 256 + (0 if h == "A" else 128):
                                  256 + (0 if h == "A" else 128) + KH]

                    for k in range(KH):
                        nc.tensor.matmul(padg[:, k:k + 1],
                                         lhsT=s0t[:, k * 128:(k + 1) * 128],
                                         rhs=adcol[layer][:, b:b + 1],
                                         start=True, stop=True)
                    adg = wp.tile([128, KH], f32, tag=f"adg{h}")
                    nc.vector.tensor_copy(out=adg[:], in_=padg[:])
                    z = wp.tile([128, KH], f32, tag=f"z{h}")
                    nc.vector.tensor_tensor(out=z[:], in0=adg[:],
                                            in1=m[:, :, 1024], op=ALU.add)
                    zs = wp.tile([128, KH], f32, tag=f"zs{h}")
                    nc.vector.tensor_scalar(out=zs[:], in0=z[:],
                                            scalar1=NEG_SLOPE, scalar2=None,
                                            op0=ALU.mult)
                    ee = wp.tile([128, KH], f32, tag=f"ee{h}")
                    nc.vector.tensor_tensor(out=ee[:], in0=z[:], in1=zs[:],
                                            op=ALU.max)
                    ex = wp.tile([128, KH], f32, tag=f"ex{h}")
                    nc.scalar.activation(ex[:], ee[:], AF.Exp)
                    for k in range(KH):
                        c = b * KH + k
                        s0 = sp.tile([128, 128], f16, tag="s0")
                        nc.vector.tensor_scalar(out=s0[:], in0=iotaF[:],
                                                scalar1=dstl_t[h][:, c:c + 1],
                                                scalar2=None, op0=ALU.is_equal)
                        s = sp.tile([128, 128], f16, tag="s")
                        nc.vector.tensor_scalar(out=s[:], in0=s0[:],
                                                scalar1=ex[:, k:k + 1],
                                                scalar2=None, op0=ALU.mult)
                        st = (h == "A" and k == 0)
                        sp_ = (h == "B" and k == KB - 1)
                        for kc in range(DKX):
                            nc.tensor.matmul(
                                numT[:, kc, :],
                                lhsT=m[:, k, kc * 128:(kc + 1) * 128],
                                rhs=s[:], start=(st and kc % 4 == 0),
                                stop=(sp_ and kc % 4 == 3))
                        nc.tensor.matmul(den[0:1, :], lhsT=ones_c[:], rhs=s[:],
                                         start=st, stop=sp_)
                # normalize + bias + relu into embT_out (features on partitions)
                dn = wp.tile([128, 128], f32, tag="dn")
                nc.vector.tensor_scalar(out=dn[0:1, :], in0=den[0:1, :],
                                        scalar1=1e-30, scalar2=None, op0=ALU.max)
                rc = wp.tile([128, 128], f32, tag="rc")
                nc.vector.reciprocal(out=rc[0:1, :], in_=dn[0:1, :])
                nc.tensor.matmul(rbc_ps[:, :], lhsT=ones_r[0:1, :],
                                 rhs=rc[0:1, :], start=True, stop=True)
                rbc = wp.tile([128, 128], f32, tag="rbc_sb")
                nc.vector.tensor_copy(out=rbc[:], in_=rbc_ps[:, :])
                tsc = wp.tile([128, DKX, 128], f32, tag="tsc")
                nc.vector.tensor_tensor(
                    out=tsc[:], in0=numT[:],
                    in1=rbc[:, None, :].broadcast_to([128, DKX, 128]),
                    op=ALU.mult)
                for kc in range(DKX):
                    nc.scalar.activation(
                        embT_out[kc][:, b * 128:(b + 1) * 128], tsc[:, kc, :],
                        AF.Relu, bias=bT[:, kc:kc + 1], scale=1.0)

            def classifier(b):
                pl = ps.tile([128, 128], f32, tag="denls", name="plog", bufs=2)
                pl = pl[:, 0:N_CLS]
                for kc in range(DKX):
                    nc.tensor.matmul(
                        pl[:], lhsT=embF[kc][:, b * 128:(b + 1) * 128],
                        rhs=FC[:, kc * N_CLS:(kc + 1) * N_CLS],
                        start=(kc == 0), stop=(kc == DKX - 1))
                lg = wp.tile([128, N_CLS], f32, tag="lg")
                nc.vector.tensor_tensor(out=lg[:], in0=pl[:], in1=fcb[:],
                                        op=ALU.add)
                mx = wp.tile([128, 1], f32, tag="mx")
                nc.vector.tensor_reduce(out=mx[:], in_=lg[:], axis=AX.X,
                                        op=ALU.max)
                lgs = wp.tile([128, N_CLS], f32, tag="lgs")
                nc.vector.tensor_scalar(out=lgs[:], in0=lg[:],
                                        scalar1=mx[:, 0:1], scalar2=None,
                                        op0=ALU.subtract)
                eg = wp.tile([128, N_CLS], f32, tag="eg")
                nc.scalar.activation(eg[:], lgs[:], AF.Exp)
                sm = wp.tile([128, 1], f32, tag="sm")
                nc.vector.tensor_reduce(out=sm[:], in_=eg[:], axis=AX.X,
                                        op=ALU.add)
                rcs = wp.tile([128, 1], f32, tag="rcs")
                nc.vector.reciprocal(out=rcs[:], in_=sm[:])
                ot = wp.tile([128, N_CLS], f32, tag="ot")
                nc.vector.tensor_scalar(out=ot[:], in0=eg[:],
                                        scalar1=rcs[:, 0:1], scalar2=None,
                                        op0=ALU.mult)
                nc.sync.dma_start(out=d_out[b * 128:(b + 1) * 128, :], in_=ot[:])

            for _ in range(reps):
                # ---- layer 1 linear + split AllGather
                for mt in range(NB):
                    def lhs1(kc, mt=mt):
                        return xT[:, kc * PADN + mt * 128: kc * PADN + (mt + 1) * 128]
                    linear(1, mt, lhs1)
                    if mt == 4 and SPLIT_AG:
                        allgather(1, "A")
                if not SPLIT_AG:
                    allgather(1, "A")
                allgather(1, "B")
                # ---- edge layer 1 interleaved with layer 2 linear
                for b in range(NB):
                    edge_block(1, b, embM, bT1)

                    def lhs2(kc, b=b):
                        return embM[kc][:, b * 128:(b + 1) * 128]
                    linear(2, b, lhs2)
                    if b == 4 and SPLIT_AG:
                        allgather(2, "A")
                if not SPLIT_AG:
                    allgather(2, "A")
                allgather(2, "B")
                # ---- edge layer 2 interleaved with classifier
                for b in range(NB):
                    edge_block(2, b, embF, bT2)
                    classifier(b)
                if debug:
                    nc.sync.dma_start(out=dbg["emb1"][:], in_=embM[0][:])
                    nc.gpsimd.dma_start(out=dbg["sh1A"][:], in_=shard[(1, "A")][:])
                    ac = wp.tile([128, NB], f32, tag="dbgac")
                    nc.vector.tensor_copy(out=ac[:], in_=adcol[1][:])
                    nc.sync.dma_start(out=dbg["adc1"][:], in_=ac[:])
    nc.compile()
    return nc


# ------------------------------------------------------------------ execution
class _Runner:
    """Cached-jit SPMD executor (axon/PJRT path of run_bass_kernel_spmd)."""

    def __init__(self, nc, n_cores=NC_):
        import jax
        from jax.sharding import Mesh, PartitionSpec
        from jax.experimental.shard_map import shard_map
        import concourse.mybir as mybir
        from concourse.bass2jax import (_bass_exec_p, install_neuronx_cc_hook,
                                        partition_id_tensor)

        install_neuronx_cc_hook()
        self.jax = jax
        self.n_cores = n_cores
        pname = nc.partition_id_tensor.name if nc.partition_id_tensor else None
        in_names, out_names, out_avals, zero_outs = [], [], [], []
        for alloc in nc.m.functions[0].allocations:
            if not isinstance(alloc, mybir.MemoryLocationSet):
                continue
            name = alloc.memorylocations[0].name
            if alloc.kind == "ExternalInput":
                if name != pname:
                    in_names.append(name)
            elif alloc.kind == "ExternalOutput":
                shape = tuple(alloc.tensor_shape)
                dtype = mybir.dt.np(alloc.dtype)
                out_names.append(name)
                out_avals.append(jax.core.ShapedArray(shape, dtype))
                zero_outs.append(np.zeros(shape, dtype))
        self.in_names, self.out_names = in_names, out_names
        self.out_avals, self.zero_outs = out_avals, zero_outs
        n_params = len(in_names)
        all_in = list(in_names) + list(out_names)
        if pname is not None:
            all_in.append(pname)

        def _body(*args):
            operands = list(args)
            if pname is not None:
                operands.append(partition_id_tensor())
            return tuple(_bass_exec_p.bind(
                *operands, out_avals=tuple(out_avals), in_names=tuple(all_in),
                out_names=tuple(out_names), lowering_input_output_aliases=(),
                sim_require_finite=True, sim_require_nnan=True, nc=nc))

        devices = jax.devices()[:n_cores]
        self.mesh = Mesh(np.asarray(devices), ("core",))
        n_outs = len(out_names)
        in_specs = (PartitionSpec("core"),) * (n_params + n_outs)
        out_specs = (PartitionSpec("core"),) * n_outs
        self.sharded = jax.jit(
            shard_map(_body, mesh=self.mesh, in_specs=in_specs,
                      out_specs=out_specs, check_rep=False),
            keep_unused=True)
        self.PartitionSpec = PartitionSpec

    def set_inputs(self, in_maps):
        jax = self.jax
        n = self.n_cores
        concat_in = [
            np.concatenate([np.asarray(in_maps[c][k]) for c in range(n)], axis=0)
            for k in self.in_names]
        concat_zero = [np.zeros((n * z.shape[0], *z.shape[1:]), z.dtype)
                       for z in self.zero_outs]
        sh = jax.sharding.NamedSharding(self.mesh, self.PartitionSpec("core"))
        self._args = [jax.device_put(a, sh) for a in concat_in + concat_zero]
        jax.block_until_ready(self._args)

    def run(self):
        return self.sharded(*self._args)

    def run_np(self):
        outs = self.jax.block_until_ready(self.run())
        n = self.n_cores
        return [
            {k: np.asarray(outs[i]).reshape(n, *self.out_avals[i].shape)[c]
             for i, k in enumerate(self.out_names)}
            for c in range(n)]


def _get_runner(key, reps=1, debug=False):
    ck = (key, reps, debug, USE_DMA_GATHER, SPLIT_AG)
    if ck not in _CACHE:
        nc = _build_program(key, reps, debug)
        _CACHE[ck] = _Runner(nc)
    return _CACHE[ck]


def kernel(**inputs):
    inputs = {k: np.asarray(v) for k, v in inputs.items()}
    assert inputs["x"].shape == (N_NODES, F_IN)
    assert inputs["edge_index"].shape == (2, N_EDGES)
    key, in_maps = _pack_inputs(**inputs)
    runner = _get_runner(key)
    runner.set_inputs(in_maps)
    res = runner.run_np()
    out = np.concatenate([res[c]["out"][:NPC] for c in range(NC_)], axis=0)
    return out.astype(np.float32)
